# revision 19
# baseline (speedup 1.0000x reference)
"""Trainium2 Bass kernel for nn_HadamardMultiplier.

Computes out = x @ M.T / sqrt(N) with M = had_K (x) H_1024 (Walsh-Hadamard),
N = 12288 = 96*128, T = 8192 tokens, sharded over 8 NeuronCores by token.

Math: with h = a*128 + b (a = 96 outer, b = 7-bit inner index),
  M = G_A (x) G_B   where  G_B = H_128 (popcount sign matrix, symmetric)
                           G_A = kron(had_K, H_8)   (96x96)
so out[t, a'*128+b'] = sum_{a,b} G_A[a',a] G_B[b',b] x[t, a*128+b].

Device pipeline (per 128-token tile), all APs contiguous, no PE transposes:
  host   : stage x as bf16 XT[a, (t, b)] = x[t, a*128+b]   (layout only)
  S1 (PE): per token t: matmul(lhsT=XT[:, t, :] (96x128), rhs=G_A.T*scale)
           -> W[b, (t, a')] fp32 PSUM, evac to bf16 SBUF      (M=128, FWL)
  S2 (PE): stationary lhsT=H_128, rhs=W[:, 512-chunks]
           -> O[b', (t, a')] fp32 PSUM, evac to bf16 SBUF     (M=128, N=512)
  host   : out[t, a'*128+b'] = O[b', t, a']  (unpermute + fp32 upcast)

The 1/sqrt(N) scale is folded into G_A.T on the host (uniform bf16 rounding
of the constant = global scale error ~1e-3, well inside the 2e-2 gate).

S2 of tile i-1 is emitted after S1 of tile i (1-tile software pipeline) so
PE never waits on the W evacuation copies. Evacuations alternate DVE/ACT.
"""

import math
from contextlib import ExitStack

import numpy as np
import ml_dtypes

T_FULL = 8192
N = 12288
NCORES = 8
TOK_PER_CORE = T_FULL // NCORES   # 1024
TILE_T = 32
NTILES = TOK_PER_CORE // TILE_T   # 32
A_DIM = 96                        # N // 128
SCALE = 1.0 / math.sqrt(float(N))


def _popcount_sign(nbits: int) -> np.ndarray:
    n = 1 << nbits
    i = np.arange(n)
    a = i[:, None] & i[None, :]
    pc = np.zeros((n, n), dtype=np.int64)
    while a.any():
        pc += a & 1
        a >>= 1
    return np.where(pc % 2 == 1, -1.0, 1.0).astype(np.float32)


def _build_nc():
    import concourse.mybir as mybir
    from concourse import bacc
    from concourse.tile import TileContext

    dt = mybir.dt
    nc = bacc.Bacc(
        "TRN2",
        target_bir_lowering=False,
        debug=False,
        enable_asserts=False,
        num_devices=NCORES,
    )
    # x, pre-transposed on host: XT[a, (tile, t, b)]
    x_d = nc.dram_tensor(
        "x", [A_DIM, TOK_PER_CORE * 128], dt.bfloat16, kind="ExternalInput"
    ).ap()
    # packed constants: [:, 0:128] H_128; [:96, 128:224] G_A.T * scale
    wb_d = nc.dram_tensor("wb", [128, 224], dt.bfloat16, kind="ExternalInput").ap()
    # out, permuted: O[b', (tile, t, a')]; host unpermutes
    out_d = nc.dram_tensor(
        "out", [128, TOK_PER_CORE * A_DIM], dt.bfloat16, kind="ExternalOutput"
    ).ap()

    TB = TILE_T * 128     # 16384 input cols per tile
    TA = TILE_T * A_DIM   # 12288 output cols per tile

    with TileContext(nc) as tc, ExitStack() as ctx:
        cpool = ctx.enter_context(tc.tile_pool(name="consts", bufs=1))
        xpool = ctx.enter_context(tc.tile_pool(name="xin", bufs=12))
        wpool = ctx.enter_context(tc.tile_pool(name="w", bufs=3))
        opool = ctx.enter_context(tc.tile_pool(name="outp", bufs=4))
        ps1 = ctx.enter_context(tc.tile_pool(name="ps1", bufs=4, space="PSUM"))
        ps2 = ctx.enter_context(tc.tile_pool(name="ps2", bufs=2, space="PSUM"))

        wb = cpool.tile([128, 224], dt.bfloat16)
        nc.scalar.dma_start(out=wb[:], in_=wb_d)
        gb_sb = wb[:, 0:128]
        ga_sb = wb[:A_DIM, 128:224]

        xts = [None] * NTILES
        ws = [None] * NTILES

        def s1(i):
            # load tile i, then W[b, (t, a')] = sum_a XT[a, (t,b)] * GA.T[a, a']
            xt = xpool.tile([A_DIM, TB], dt.bfloat16)
            nc.sync.dma_start(out=xt[:], in_=x_d[:, i * TB : (i + 1) * TB])
            xts[i] = xt
            w = wpool.tile([128, TA], dt.bfloat16)
            ws[i] = w
            for g in range(TILE_T // 4):  # 4 tokens per PSUM bank
                ps = ps1.tile([128, 4 * A_DIM], dt.float32)
                for k in range(4):
                    t = g * 4 + k
                    nc.tensor.matmul(
                        ps[:, k * A_DIM : (k + 1) * A_DIM],
                        lhsT=xt[:, t * 128 : (t + 1) * 128],
                        rhs=ga_sb,
                        start=True,
                        stop=True,
                    )
                dst = w[:, g * 4 * A_DIM : (g + 1) * 4 * A_DIM]
                if (g + i) % 2 == 0:
                    nc.vector.tensor_copy(dst, ps[:])
                else:
                    nc.scalar.copy(dst, ps[:])

        def s2(i):
            # O[b', (t, a')] = sum_b H128[b, b'] * W[b, (t, a')]
            w = ws[i]
            ot = opool.tile([128, TA], dt.bfloat16)
            for j in range(TA // 1024):   # 2 PSUM banks per evacuation copy
                ps = ps2.tile([128, 1024], dt.float32)
                for h in range(2):
                    nc.tensor.matmul(
                        ps[:, h * 512 : (h + 1) * 512],
                        lhsT=gb_sb,
                        rhs=w[:, (2 * j + h) * 512 : (2 * j + h + 1) * 512],
                        start=True,
                        stop=True,
                    )
                # split the evacuation across both engines in parallel
                c0 = j * 1024
                if (j + i) % 2 == 0:
                    nc.vector.tensor_copy(ot[:, c0 : c0 + 512], ps[:, 0:512])
                    nc.scalar.copy(ot[:, c0 + 512 : c0 + 1024], ps[:, 512:1024])
                else:
                    nc.scalar.copy(ot[:, c0 : c0 + 512], ps[:, 0:512])
                    nc.vector.tensor_copy(ot[:, c0 + 512 : c0 + 1024], ps[:, 512:1024])
            # late tiles: ACT's queue is drained by then; HWDGE dispatch is prompt
            if i >= NTILES - 3:
                nc.scalar.dma_start(out=out_d[:, i * TA : (i + 1) * TA], in_=ot[:])
            else:
                nc.gpsimd.dma_start(out=out_d[:, i * TA : (i + 1) * TA], in_=ot[:])

        s1(0)
        for i in range(1, NTILES):
            s1(i)
            s2(i - 1)
        s2(NTILES - 1)
    nc.compile()
    return nc


_NC_CACHE = None


def _get_nc():
    global _NC_CACHE
    if _NC_CACHE is None:
        _NC_CACHE = _build_nc()
    return _NC_CACHE


def _make_weight_input(had_K: np.ndarray) -> np.ndarray:
    bf16 = ml_dtypes.bfloat16
    h128 = _popcount_sign(7)
    h8 = _popcount_sign(3)
    ga_t = np.kron(had_K.astype(np.float32), h8).T * np.float32(SCALE)
    wb = np.zeros((128, 224), dtype=np.float32)
    wb[:, 0:128] = h128
    wb[:A_DIM, 128:224] = ga_t
    return wb.astype(bf16)


def run(x: np.ndarray, had_K: np.ndarray, trace: bool = False):
    """Run the kernel; returns (out, BassKernelResults)."""
    from concourse.bass_utils import run_bass_kernel_spmd

    bf16 = ml_dtypes.bfloat16
    x = np.asarray(x, dtype=np.float32)
    had_K = np.asarray(had_K, dtype=np.float32)
    assert x.shape == (T_FULL, N), x.shape
    wb = _make_weight_input(had_K)

    nc = _get_nc()
    in_maps = []
    for c in range(NCORES):
        shard = x[c * TOK_PER_CORE : (c + 1) * TOK_PER_CORE]
        # XT[a, (t, b)] = shard[t, a*128+b], bf16
        xt = np.ascontiguousarray(
            shard.reshape(TOK_PER_CORE, A_DIM, 128).transpose(1, 0, 2)
        ).astype(bf16)
        in_maps.append({"x": xt.reshape(A_DIM, TOK_PER_CORE * 128), "wb": wb})

    res = run_bass_kernel_spmd(nc, in_maps, core_ids=list(range(NCORES)), trace=trace)
    outs = []
    for r in res.results:
        o = np.asarray(r["out"])  # [128, TOK_PER_CORE * A_DIM] bf16
        o = o.reshape(128, TOK_PER_CORE, A_DIM).transpose(1, 2, 0)  # [t, a', b']
        outs.append(o.reshape(TOK_PER_CORE, N).astype(np.float32))
    out = np.concatenate(outs, axis=0)
    return out, res


def kernel(x: np.ndarray, had_K: np.ndarray) -> np.ndarray:
    out, _ = run(x, had_K, trace=False)
    return out.astype(np.float32)
